# revision 1
# baseline (speedup 1.0000x reference)
"""Trainium2 Bass kernel for nn_CustomLoss_51677046505531.

loss = 0.5 * mean((logits-labels)^2)
     + 0.5 * sum_{labels_i > labels_j} relu(1 - (logits_i - logits_j)) / #pairs

Strategy
--------
Host: argsort by labels. With g = logits sorted by label ascending, the masked
pairwise sum becomes a strict lower-triangle sum over positions:
    sum_{a > b} relu((1 + g_b) - g_a)
(ties corrected exactly on host; #pairs computed exactly on host).

Device (8 cores, SPMD one program, rows/cols interleaved idx%8==core):
  The triangle of 64x64 [128,128] blocks is split three ways:
  * col-blocks q < K      -> ScalarE: one Relu activation (scale=-1,
      per-partition bias=1+g_col, fused accum_out) per q over gbig, a bf16
      broadcast-DMA of the core's interleaved rows.
  * col-blocks q >= K     -> VectorE tensor_scalar ((in0 - scalar) max 0,
      bf16 fast mode) producing hinge tiles from tbig (bf16 broadcast of
      1+g cols), reduced by TensorE bf16 ones-matmuls (1 cycle/row) into a
      [1,512] PSUM accumulator; partitions = 128 rows of row-block b.
      Row-jobs ascend so the first ones only need the head tbig chunk; a
      dummy zero matmul opens the PSUM group with no input dependency.
  * diagonal blocks: host sends pre-masked bf16 pre-hinge values; relu'd
      on VectorE (max(x,0)) and folded into the same PE reduction.
  A dummy activation at t=0 preloads the Relu table; broadcast DMA chunks
  are ordered ahead of the small loads so compute starts ~2.5us in.
  MSE partials on VectorE. Final tiny gather/combine on host.
bf16 is used for all pairwise operands (inputs are ~N(0,1); the 2e-2
tolerance leaves ~100x headroom over bf16 rounding noise). The fused
accum_out path on VectorE and all gpsimd compute are avoided: both are
far slower on real silicon than the cost model suggests (and gpsimd
elementwise ops don't compile on this backend).
"""

import sys

sys.path.insert(0, "/opt/trn_rl_repo")

from contextlib import ExitStack

import numpy as np
import ml_dtypes

import concourse.bass as bass
import concourse.tile as tile
from concourse import mybir
from concourse.bacc import Bacc
from concourse.bass_utils import run_bass_kernel_spmd

ALPHA = 0.5
N = 8192
NCORES = 8
P = 128
B = N // P            # 64 blocks of 128
NPC = N // NCORES     # 1024 interleaved elements per core
K = 9                # col-blocks on ScalarE; col-blocks >= K go to VectorE+PE
DIAG_PER_CORE = B // NCORES  # 8
BIG_NEG = -1.0e30
F32 = mybir.dt.float32
BF16 = mybir.dt.bfloat16
BF16NP = ml_dtypes.bfloat16

_CACHE = {}


def _build_nc(reps=1, skip_act=False, skip_dve=False, skip_bcast=False, skip_pe=False):
    """Build the SPMD program. reps>1 wraps the compute in a For_i loop for
    slope-based wall-clock timing (NTFF profiling is unavailable here).
    skip_* flags are for ablation timing only (wrong results)."""
    nc = Bacc()
    t_c = nc.declare_dram_parameter("t_c", [1, NPC], BF16, isOutput=False)
    g_c = nc.declare_dram_parameter("g_c", [1, NPC], BF16, isOutput=False)
    grows = nc.declare_dram_parameter("g_rows", [P, B], F32, isOutput=False)
    tcols = nc.declare_dram_parameter("t_cols", [P, K], F32, isOutput=False)
    d_pre = nc.declare_dram_parameter(
        "d_pre", [P, DIAG_PER_CORE * P], BF16, isOutput=False
    )
    mse_x = nc.declare_dram_parameter("mse_x", [P, N // NCORES // P], F32, isOutput=False)
    mse_y = nc.declare_dram_parameter("mse_y", [P, N // NCORES // P], F32, isOutput=False)
    out_acc = nc.declare_dram_parameter("out_acc", [P, K + 2], F32, isOutput=True)
    out_pe = nc.declare_dram_parameter("out_pe", [1, 512], F32, isOutput=True)

    relu = mybir.ActivationFunctionType.Relu
    alu = mybir.AluOpType

    with ExitStack() as ctx:
        tc = ctx.enter_context(tile.TileContext(nc))
        const = ctx.enter_context(tc.tile_pool(name="const", bufs=1))
        prods = ctx.enter_context(tc.tile_pool(name="prods", bufs=6))
        psum = ctx.enter_context(tc.tile_pool(name="psum", bufs=1, space="PSUM"))

        # ---- small input loads on the ACT queue so the SP queue is free
        # for the broadcast chunks that gate the first compute. grows first
        # (DVE's per-partition scalars), then ACT's own inputs, mse last.
        grows_s = const.tile([P, B], F32)
        tcols_s = const.tile([P, K], F32)
        dpre_s = const.tile([P, DIAG_PER_CORE * P], BF16)
        msex_s = const.tile([P, N // NCORES // P], F32)
        msey_s = const.tile([P, N // NCORES // P], F32)
        nc.scalar.dma_start(out=grows_s, in_=grows[:, :])
        nc.scalar.dma_start(out=tcols_s, in_=tcols[:, :])

        ones_col = const.tile([P, 1], BF16)
        nc.gpsimd.memset(ones_col, 1.0)
        zer512 = const.tile([P, 512], BF16)
        nc.gpsimd.memset(zer512, 0.0)
        warm = const.tile([P, 1], BF16)
        nc.gpsimd.memset(warm, 0.0)
        warm2 = const.tile([P, 1], BF16)
        nc.scalar.activation(out=warm2, in_=warm, func=relu, bias=0.0, scale=1.0)

        gbig_p = const.tile([P, NPC], BF16)
        tbig_s = const.tile([P, NPC], BF16)
        acc_s = const.tile([P, K + 2], F32)
        scr_a = const.tile([P, NPC], BF16)
        scr_d = const.tile([P, DIAG_PER_CORE * P], BF16)
        nmse = N // NCORES // P
        dif = const.tile([P, nmse], F32)
        sqo = const.tile([P, nmse], F32)
        pe_acc = psum.tile([1, 512], F32)
        nc.vector.memset(acc_s, 0.0)

        # Broadcast chunks ordered for earliest compute start: tbig head
        # (ascending DVE row-jobs read cols from 16K upward), then gbig
        # (ACT jobs need all rows), then the remaining tbig chunks, then
        # the late-consumed inputs (diag band, mse).
        if not skip_bcast:
            nc.sync.dma_start(
                out=tbig_s[:, :256], in_=t_c[:, :256].to_broadcast([P, 256])
            )
            for h in range(0, NPC, 512):
                nc.sync.dma_start(
                    out=gbig_p[:, h : h + 512],
                    in_=g_c[:, h : h + 512].to_broadcast([P, 512]),
                )
            for h in range(256, NPC, 256):
                nc.sync.dma_start(
                    out=tbig_s[:, h : h + 256],
                    in_=t_c[:, h : h + 256].to_broadcast([P, 256]),
                )
        nc.sync.dma_start(out=dpre_s, in_=d_pre[:, :])
        nc.sync.dma_start(out=msex_s, in_=mse_x[:, :])
        nc.sync.dma_start(out=msey_s, in_=mse_y[:, :])

        def emit_compute():


            # -- ScalarE: col-blocks q < K (suffix rows, fused accum) -----
            for q in range(K) if not skip_act else []:
                lo = 16 * (q + 1)
                nc.scalar.activation(
                    out=scr_a[:, : NPC - lo],
                    in_=gbig_p[:, lo:NPC],
                    func=relu,
                    bias=tcols_s[:, q : q + 1],
                    scale=-1.0,
                    accum_out=acc_s[:, q : q + 1],
                )


            # -- VectorE + PE: row-blocks b > K over cols [16K, 16b) ------
            # Ascending so the first jobs only need the tbig head chunk; a
            # dummy full-width matmul on zeros opens (and zeroes) the PSUM
            # accumulation region with no input dependency.
            if not skip_pe:
                nc.tensor.matmul(
                    pe_acc[:, :512], lhsT=ones_col, rhs=zer512,
                    start=True, stop=False,
                )
            bs = list(range(K + 1, B)) if not skip_dve else []
            for b in bs:
                ext = 16 * (b - K)
                prod = prods.tile([P, ext], BF16, tag="prod")
                nc.vector.tensor_scalar(
                    out=prod[:, :ext],
                    in0=tbig_s[:, 16 * K : 16 * b],
                    scalar1=grows_s[:, b : b + 1],
                    scalar2=0.0,
                    op0=alu.subtract,
                    op1=alu.max,
                )
                for off in range(0, ext, 512) if not skip_pe else []:
                    w = min(512, ext - off)
                    nc.tensor.matmul(
                        pe_acc[:, :w],
                        lhsT=ones_col,
                        rhs=prod[:, off : off + w],
                        start=False,
                        stop=False,
                    )
            # diagonal blocks (host pre-masked): relu is max(x, 0), same
            # DVE + PE-reduce pipeline; closes the PSUM group.
            if not skip_dve:
                dd = DIAG_PER_CORE * P
                nc.vector.tensor_scalar(
                    out=scr_d,
                    in0=dpre_s,
                    scalar1=0.0,
                    scalar2=0.0,
                    op0=alu.max,
                    op1=alu.max,
                )
                for off in range(0, dd, 512) if not skip_pe else []:
                    w = min(512, dd - off)
                    nc.tensor.matmul(
                        pe_acc[:, :w],
                        lhsT=ones_col,
                        rhs=scr_d[:, off : off + w],
                        start=False,
                        stop=off + w >= dd,
                    )

            # -- VectorE: MSE partials last (their inputs land last) ------
            nc.vector.tensor_sub(dif, msex_s, msey_s)
            nc.vector.scalar_tensor_tensor(
                out=sqo,
                in0=dif,
                scalar=0.0,
                in1=dif,
                op0=alu.bypass,
                op1=alu.mult,
                accum_out=acc_s[:, K + 1 : K + 2],
            )

        if reps > 1:
            with tc.For_i(0, reps, 1):
                emit_compute()
        else:
            emit_compute()

        # ---- outputs ----------------------------------------------------
        pe_stage = const.tile([1, 512], F32)
        if skip_dve or skip_pe:
            nc.vector.memset(pe_stage, 0.0)
        else:
            nc.scalar.copy(out=pe_stage, in_=pe_acc)
        nc.sync.dma_start(out=out_acc[:, :], in_=acc_s)
        nc.sync.dma_start(out=out_pe[:, :], in_=pe_stage)

    # Bacc.finalize runs alloc_regs + generate_event_semaphores (splits
    # multi-sem waits that the PE ISA can't encode); run_bass_via_pjrt
    # doesn't call it for prebuilt modules.
    nc.finalize()
    return nc


def _host_prep(logits, labels):
    """Sort by labels, build per-core input maps + exact host-side scalars."""
    logits = np.asarray(logits, dtype=np.float32).reshape(N)
    labels = np.asarray(labels, dtype=np.float32).reshape(N)
    order = np.argsort(labels, kind="stable")
    g = np.ascontiguousarray(logits[order]).astype(np.float32)
    labs = labels[order]
    T = (1.0 + g).astype(np.float32)

    # Exact #pairs with labels_i > labels_j, and the correction for tie pairs
    # that the device's positional triangle wrongly includes.
    num_pairs = N * (N - 1) // 2
    tie_corr = 0.0
    change = np.nonzero(np.diff(labs))[0] + 1
    starts = np.concatenate([[0], change])
    ends = np.concatenate([change, [N]])
    for a, e in zip(starts, ends):
        m = int(e - a)
        if m > 1:
            num_pairs -= m * (m - 1) // 2
            gg = g[a:e].astype(np.float64)
            d = 1.0 + gg[None, :] - gg[:, None]  # [i, j] = 1 + g_j - g_i
            tie_corr += float(np.maximum(d, 0.0)[np.tril_indices(m, -1)].sum())

    grows = np.ascontiguousarray(g.reshape(B, P).T)           # [P, B]
    tcols = np.ascontiguousarray(T.reshape(B, P).T[:, :K])    # [P, K]

    il = np.tril_indices(P, -1)
    in_maps = []
    for c in range(NCORES):
        dm = np.full((P, DIAG_PER_CORE, P), BIG_NEG, dtype=np.float32)
        for d in range(DIAG_PER_CORE):
            blk = DIAG_PER_CORE * c + d
            gg = g[P * blk : P * (blk + 1)]
            pre = (1.0 + gg[None, :] - gg[:, None]).astype(np.float32)
            dm[il[0], d, il[1]] = pre[il]
        in_maps.append(
            {
                "t_c": np.ascontiguousarray(T[c::NCORES]).reshape(1, NPC).astype(BF16NP),
                "g_c": np.ascontiguousarray(g[c::NCORES]).reshape(1, NPC).astype(BF16NP),
                "g_rows": grows,
                "t_cols": tcols,
                "d_pre": np.ascontiguousarray(dm.reshape(P, DIAG_PER_CORE * P)).astype(BF16NP),
                "mse_x": np.ascontiguousarray(logits[c::NCORES].reshape(P, -1)),
                "mse_y": np.ascontiguousarray(labels[c::NCORES].reshape(P, -1)),
            }
        )
    return in_maps, num_pairs, tie_corr


def _combine(results, num_pairs, tie_corr):
    rank_dev = 0.0
    sse = 0.0
    for c in range(NCORES):
        oa = results[c]["out_acc"].astype(np.float64)
        op = results[c]["out_pe"].astype(np.float64)
        rank_dev += oa[:, : K + 1].sum() + op.sum()
        sse += oa[:, K + 1].sum()
    rank_sum = rank_dev - tie_corr
    mse = sse / N
    ranking = rank_sum / max(num_pairs, 1) if num_pairs > 0 else 0.0
    return np.float32(ALPHA * mse + (1.0 - ALPHA) * ranking)


def kernel(logits, labels, **_unused):
    in_maps, num_pairs, tie_corr = _host_prep(logits, labels)
    if "nc" not in _CACHE:
        _CACHE["nc"] = _build_nc()
    # two executions; keep the second (guards against first-run engine/DMA
    # state flakiness observed on this stack)
    run_bass_kernel_spmd(_CACHE["nc"], in_maps, list(range(NCORES)))
    res = run_bass_kernel_spmd(_CACHE["nc"], in_maps, list(range(NCORES)))
    return _combine(res.results, num_pairs, tie_corr)

